# revision 23
# baseline (speedup 1.0000x reference)
"""LpAlignEntropyLoss Trainium2 kernel (8 NeuronCores, SPMD).

loss = mean_i ||v0_i - v1_i||_2                                (align, host)
     + 0.5*sum_views mean_i [ln S_i - ln(N-1)],  S_i = sum_{j!=i} exp(-d_ij)

Symmetric "tournament" scheme (halves the O(N^2) work):
  N=8192 rows = 64 blocks of 128. Core c receives z ROTATED by -1024c rows,
  so its own rows are local rows 0..1023. Local row-tile k (rows 128k..+128)
  computes pairwise distances against local cols [128k, 128k+4224) -- its own
  block plus the 32 blocks "ahead" (mod 64 globally, static locally thanks to
  the rotation). Every unordered pair lands in exactly one tile:
    distance  0 block: full, diag masked (+BIG), row-sums only
    distance 1..31   : row-sums + column-sums
    distance 32 block: computed from BOTH sides, row-sums only
  Device ships per-tile row sums [128] and column partials [3968] (f32);
  host un-rotates, sums partials across cores, takes ln in f64, and adds the
  host-computed align term.

The PE clock gate (HAM) throttles to 1.2 GHz whenever low-utilization
matmuls (k=1 / m=1) pollute the stream, so sq_i/sq_j enter the GEMM as two
augmented contraction features instead of rank-1 updates: K=256 z-dims + 2
aug rows split into 3 k-groups of 86.  psum = z_i.z_j + (256-sq_i)/2 +
(256-sq_j)/2, so d2 = -2*psum + 512 with a constant ACT bias.  Column sums
use full-width m=128 ones matmuls (all rows identical; row 0 shipped).
"""

import sys

for _p in ("/opt/trn_rl_repo",):
    if _p not in sys.path:
        sys.path.insert(0, _p)

import math

import ml_dtypes
import numpy as np

import concourse.bass as bass
from concourse import bacc
import concourse.mybir as mybir
import concourse.tile as tile
from concourse.bass import ds, ts
from concourse.tile import add_dep_helper

F32 = mybir.dt.float32
BF16 = mybir.dt.bfloat16
AF = mybir.ActivationFunctionType

N = 8192          # rows per view
K = 256           # features
NCORES = 8
R = N // NCORES   # rows per core = 1024
NT = 8            # row tiles per core
TW = 4224         # cols per tile  (33 blocks of 128)
CSW = 3968        # col-sum region width (31 blocks: skip own + distance-32)
BIG = 30000.0     # diag d2 offset -> exp(-sqrt(BIG)) == 0 in f32
LCH = 512         # load chunk (f32 [128, LCH])
NLCH = 64 * K // LCH
TCW = 4096        # dma transpose piece: [TCW, 128] -> [128, TCW]
KG = ((0, 86), (86, 86), (172, 84))   # k-group (z-offset, n z-dims)


def build_nc():
    nc = bacc.Bacc()

    vb_in = [
        nc.declare_dram_parameter("vb0", [N, K], BF16, isOutput=False),
        nc.declare_dram_parameter("vb1", [N, K], BF16, isOutput=False),
    ]
    sqc_in = [
        nc.declare_dram_parameter("sqc0", [1, N], BF16, isOutput=False),
        nc.declare_dram_parameter("sqc1", [1, N], BF16, isOutput=False),
    ]
    onesr_in = nc.declare_dram_parameter("onesrow", [1, N], BF16, isOutput=False)
    zeros_in = nc.declare_dram_parameter("zeros", [126, N], BF16, isOutput=False)
    eye_in = nc.declare_dram_parameter("eye", [128, 128], BF16, isOutput=False)
    nbe_in = nc.declare_dram_parameter("negbigeye", [128, 128], BF16, isOutput=False)
    srow_ext = nc.declare_dram_parameter("srow", [128, 2 * NT], F32, isOutput=True)
    colp_ext = nc.declare_dram_parameter("colp", [2 * NT, CSW], F32, isOutput=True)

    with tile.TileContext(nc) as tc:
        with (
            tc.tile_pool(name="consts", bufs=1) as consts,
            tc.tile_pool(name="persist", bufs=1) as persist,
            tc.tile_pool(name="zt", bufs=2) as ztp,
            tc.tile_pool(name="ztq", bufs=2) as ztqp,
            tc.tile_pool(name="dpool", bufs=8) as dpool,
            tc.tile_pool(name="epool", bufs=3) as epool,
            tc.tile_pool(name="mmps", bufs=2, space="PSUM") as mmps,
            tc.tile_pool(name="auxps", bufs=4, space="PSUM") as auxps,
            tc.tile_pool(name="cstage", bufs=6) as cstp,
            tc.tile_pool(name="dram", bufs=2, space="DRAM") as dramp,
        ):
            # ---------------- constants ----------------
            eye_sb = consts.tile([128, 128], BF16, name="eye_sb")
            nc.sync.dma_start(out=eye_sb, in_=eye_in[:, :])
            nbe_sb = consts.tile([128, 128], BF16, name="nbe_sb")
            nc.sync.dma_start(out=nbe_sb, in_=nbe_in[:, :])
            ones128 = consts.tile([128, 128], BF16, name="ones128")
            nc.vector.memset(ones128, 1.0)
            wdum = consts.tile([128, 512], BF16, name="wdum")
            nc.vector.memset(wdum, 0.0)
            b512 = consts.tile([128, 1], F32, name="b512")
            nc.vector.memset(b512, 512.0)

            S_sb = persist.tile([128, 2 * NT], F32, name="S_sb")

            # ---------------- z load + transpose (per view) ----------------
            # v [8192, 256] f32 --load--> sbuf f32 chunks --DVE--> bf16
            # --store--> scrz [8192, 256] bf16 --dma_transpose--> 3 k-group
            # tiles [86, 8192] bf16; group 2 rows 84/85 carry aug features.
            def load_view(v):
                # kt tiles straight from the bf16 DRAM input (k=128 each);
                # group 2 = [ones; sqc; 126 zero rows] so every matmul in
                # the PE stream is full-utilization (keeps HAM un-throttled)
                ztA = ztp.tile([128, N], BF16, name=f"ztA{v}", tag="ztA")
                ztB = ztp.tile([128, N], BF16, name=f"ztB{v}", tag="ztB")
                ztG = ztp.tile([128, N], BF16, name=f"ztG{v}", tag="ztG")
                ztq = ztqp.tile([128, R], BF16, name=f"ztq{v}", tag="ztq")
                nc.sync.dma_start(out=ztG[ds(0, 1), :], in_=onesr_in[:, :])
                nc.sync.dma_start(out=ztG[ds(1, 1), :], in_=sqc_in[v][:, :])
                nc.sync.dma_start(out=ztG[ds(2, 126), :], in_=zeros_in[:, :])
                nc.sync.dma_start(out=ztq[ds(0, 1), :], in_=sqc_in[v][:, ds(0, R)])
                nc.sync.dma_start(out=ztq[ds(1, 1), :], in_=onesr_in[:, ds(0, R)])
                nc.sync.dma_start(out=ztq[ds(2, 126), :], in_=zeros_in[:, ds(0, R)])
                tpg = None
                for c in range(N // TCW):
                    ti = nc.sync.dma_start_transpose(
                        ztA[:, ts(c, TCW)], vb_in[v][ts(c, TCW), ts(0, 128)])
                    if c == 1:
                        tpg = ti
                    nc.sync.dma_start_transpose(
                        ztB[:, ts(c, TCW)], vb_in[v][ts(c, TCW), ts(1, 128)])
                return [ztA, ztB, ztG], ztq, tpg

            # ---------------- per-tile GEMM -> psum chunks ----------------
            CHUNKS = [(0, 1024), (1024, 1024), (2048, 1024), (3072, 1024), (4096, 128)]

            def gemm_tile(zt, ztq2, k, act_out):
                lo = 128 * k
                lhs = [zt[0], zt[1], ztq2]
                for off, w in CHUNKS:
                    ps = mmps.tile([128, 1024], F32, name="mm", tag="mm")
                    nblk = [(n0, min(512, w - n0)) for n0 in range(0, w, 512)]
                    for g in range(3):
                        lt = lhs[g][:, ds(lo, 128)]
                        for n0, nw in nblk:
                            diag0 = off == 0 and n0 == 0
                            nc.tensor.matmul(
                                ps[:, ds(n0, nw)], lt,
                                zt[g][:, ds(lo + off + n0, nw)],
                                start=(g == 0), stop=(g == 2 and not diag0),
                                skip_group_check=(g > 0),
                            )
                    if off == 0:
                        nc.tensor.matmul(
                            ps[:, ds(0, 128)], nbe_sb, eye_sb,
                            start=False, stop=True, skip_group_check=True,
                        )
                    act_out(ps, off, w)

            # ---------------- main schedule ----------------
            sqrt_w = {0: [], 1: []}
            exp_w = {0: [], 1: []}

            zt_v, ztq_v = {}, {}
            zt_v[0], ztq_v[0], tp0 = load_view(0)
            zt_v[1], ztq_v[1], _ = load_view(1)

            # HAM warmup timed to land right before the first GEMM
            warm = []
            for i in range(10):
                wps = mmps.tile([128, 1024], F32, name="mm", tag="mm")
                wi = nc.tensor.matmul(
                    wps[:, ds(0, 512)], wdum[:, ds(0, 128)], wdum,
                    start=True, stop=True,
                )
                warm.append(wi)
            add_dep_helper(warm[0].ins, tp0.ins, False, "warmup after first tp")

            phase_lists = []
            for v, k0, k1 in ((0, 0, NT), (1, 0, 4), (1, 4, NT)):
                sq_list, ex_list = [], []
                d_tiles = {}
                for k in range(k0, k1):
                    dt_ = dpool.tile([128, TW], BF16, name=f"d{v}{k}", tag="d")
                    d_tiles[k] = dt_

                    def p1(ps, off, w, dt_=dt_, sq_list=sq_list):
                        si = nc.scalar.activation(
                            out=dt_[:, ds(off, w)], in_=ps[:, ds(0, w)],
                            func=AF.Sqrt, bias=b512, scale=-2.0,
                        )
                        sq_list.append(si)

                    gemm_tile(zt_v[v], ztq_v[v], k, p1)

                for k in range(k0, k1):
                    et = epool.tile([128, TW], BF16, name="e", tag="e")
                    ei = nc.scalar.activation(
                        out=et, in_=d_tiles[k], func=AF.Exp, scale=-1.0,
                        accum_out=S_sb[:, ds(v * NT + k, 1)],
                    )
                    ex_list.append(ei)
                    # column sums over E[:, 128:4096] -> colp row v*NT+k
                    # (m=128 ones matmul: all psum rows identical, ship row 0)
                    for n0 in range(0, CSW, 512):
                        nw = min(512, CSW - n0)
                        cs = auxps.tile([128, 512], F32, name="cs", tag="cs")
                        nc.tensor.matmul(
                            cs[:, ds(0, nw)], ones128, et[:, ds(128 + n0, nw)],
                            start=True, stop=True,
                        )
                        stg = cstp.tile([128, 512], F32, name="cst", tag="cst")
                        nc.vector.tensor_copy(stg[:, ds(0, nw)], cs[:, ds(0, nw)])
                        nc.sync.dma_start(
                            out=colp_ext[ds(v * NT + k, 1), ds(n0, nw)],
                            in_=stg[ds(0, 1), ds(0, nw)],
                        )
                phase_lists += [sq_list, ex_list]

            nc.sync.dma_start(out=srow_ext[:, :], in_=S_sb)

            # ACT phase ordering: alternate sqrt/exp windows per segment
            for a, b in zip(phase_lists, phase_lists[1:]):
                if a and b:
                    add_dep_helper(b[0].ins, a[-1].ins, True, "act phase order")

    nc.finalize()
    return nc


_NC = None


def _get_nc():
    global _NC
    if _NC is None:
        _NC = build_nc()
    return _NC


def _in_maps(v0, v1):
    v0 = np.ascontiguousarray(v0, dtype=np.float32)
    v1 = np.ascontiguousarray(v1, dtype=np.float32)
    eye = np.eye(128, dtype=ml_dtypes.bfloat16)
    nbe = ((-BIG / 2.0) * np.eye(128, dtype=np.float32)).astype(ml_dtypes.bfloat16)
    onesrow = np.ones((1, N), dtype=ml_dtypes.bfloat16)
    zeros = np.zeros((126, N), dtype=ml_dtypes.bfloat16)
    maps = []
    for c in range(NCORES):
        m = {"eye": eye, "negbigeye": nbe, "onesrow": onesrow, "zeros": zeros}
        for v, arr in ((0, v0), (1, v1)):
            vrot = np.roll(arr, -R * c, axis=0)
            sq = np.einsum("ij,ij->i", vrot.astype(np.float64), vrot.astype(np.float64))
            m[f"vb{v}"] = vrot.astype(ml_dtypes.bfloat16)
            m[f"sqc{v}"] = ((256.0 - sq) / 2.0).astype(ml_dtypes.bfloat16)[None, :]
        maps.append(m)
    return maps


def _combine(results, v0, v1):
    ent = 0.0
    for v in (0, 1):
        S = np.zeros(N, np.float64)
        for c, res in enumerate(results):
            srow = res["srow"]          # [128, 16]
            colp = res["colp"]          # [16, 3968]
            Sl = np.zeros(N, np.float64)
            for k in range(NT):
                Sl[128 * k:128 * k + 128] += srow[:, v * NT + k]
                Sl[128 * k + 128:128 * k + 128 + CSW] += colp[v * NT + k]
            S += np.roll(Sl, R * c)
        ent += (np.log(S) - math.log(N - 1)).mean()
    d = v0.astype(np.float64) - v1.astype(np.float64)
    align = np.sqrt((d * d).sum(1)).mean()
    return np.float32(align + ent / 2.0)


def run_device(v0, v1, trace=False):
    from concourse.bass_utils import run_bass_kernel_spmd

    nc = _get_nc()
    res = run_bass_kernel_spmd(
        nc, _in_maps(v0, v1), core_ids=list(range(NCORES)), trace=trace
    )
    return res


def kernel(v0, v1):
    res = run_device(v0, v1, trace=False)
    return _combine(res.results, v0, v1)


if __name__ == "__main__":
    rng = np.random.default_rng(0)
    v0 = rng.standard_normal((N, K), dtype=np.float32)
    v1 = rng.standard_normal((N, K), dtype=np.float32)
    print("building...")
    nc = _get_nc()
    print("running...")
    out = kernel(v0, v1)
    print("loss:", out)
